# revision 61
# baseline (speedup 1.0000x reference)
"""Multi-head attention block (B=4, N=1024, C=1024, H=16, d=64) on 8 TRN2 cores.

Sharding: core = 2*b + hh  (batch b in 0..3, head-half hh in 0..1 -> 8 heads/core).
Each core computes the qkv projection for its 8 heads, attention, and a partial
output projection (its 512 rows of w_proj). Host sums the two partials per
batch and adds b_proj.

Per-core pipeline (dense matmuls in float32r -> 1 PE cycle/row; softmax
probabilities and V in bf16, accumulation always f32 in PSUM):
  - x^T fed from host, so Y_qk^T[cols,seq] = (W_qk strips).T @ x^T chunks gives
    q^T,k^T directly; Y_v[seq,vcols] = (x^T chunks).T @ W_v gives V naturally.
    Zero on-chip transposes.
  - per head pair (heads 2p, 2p+1 at partition bases 0/64): S^T[keys,q] =
    k^T.T @ q^T with K=64; the two heads' matmuls target disjoint PE row
    groups. exp on ACT (softmax max-subtraction safely skipped; the 1/sqrt(d)
    scale is folded into the exp), output bf16.
  - AV with V augmented by a ones column: one PSUM accumulation yields both
    att^T[64,q] and the softmax denominators (row 64). Normalize: DVE
    reciprocal (PSUM row), gpsimd partition_broadcast, DVE multiply -> att^T.
  - proj: out[seq,outfeat] = (att^T chunks).T @ w_proj chunks, DVE evict,
    DMA out.
The emission order software-pipelines the stages: while ACT runs pair p's
exp stream, PE retires AV groups of pair p-1 (Y_v for p=0) and the next
pair's QKV matmuls; the final AV groups interleave with the projection.
"""

import numpy as np

B = 4
N = 1024
C = 1024
H = 16
D = 64
NCORES = 8
SCALE = D ** -0.5


_NC_CACHE = {}


def _build_bass():
    import concourse.mybir as mybir
    from concourse import bacc
    from concourse.tile import TileContext

    dt = mybir.dt
    f32 = dt.float32
    f32r = dt.float32r
    Act = mybir.ActivationFunctionType

    nc = bacc.Bacc(
        "TRN2",
        target_bir_lowering=False,
        debug=False,
        num_devices=NCORES,
        num_swdge_queues=4,
    )

    # ---- DRAM I/O (per-core shards; host prepares layouts) ----
    xT_d = nc.dram_tensor("xT", [C, N], f32r, kind="ExternalInput").ap()
    wqk_d = nc.dram_tensor("wqk", [8, 128, 8, 128], f32r, kind="ExternalInput").ap()
    wv_d = nc.dram_tensor("wv", [C, 512], f32r, kind="ExternalInput").ap()
    wp_d = nc.dram_tensor("wp", [512, C], f32r, kind="ExternalInput").ap()
    bqk_d = nc.dram_tensor("bqk", [128, 8], f32, kind="ExternalInput").ap()
    bv_d = nc.dram_tensor("bv", [128, 512], f32, kind="ExternalInput").ap()
    ones_d = nc.dram_tensor("ones64", [128, 64], dt.bfloat16, kind="ExternalInput").ap()
    y_d = nc.dram_tensor("y", [N, C], f32, kind="ExternalOutput").ap()

    with TileContext(nc) as tc:
        with (
            tc.tile_pool(name="persist", bufs=1) as persist,
            tc.tile_pool(name="yqk_pool", bufs=3) as yqk_pool,
            tc.tile_pool(name="es_pool", bufs=22) as es_pool,
            tc.tile_pool(name="norm", bufs=3) as norm,
            tc.tile_pool(name="psum", bufs=2, space="PSUM") as ps,
            tc.tile_pool(name="psum_sm", bufs=2, space="PSUM") as ps_sm,
            tc.tile_pool(name="psav", bufs=2, space="PSUM") as psav,
        ):
            # persistent SBUF tensors
            vst = persist.tile([128, 8, 8, 65], dt.bfloat16, tag="vst")  # [keys128, s, h, d+1]
            attr = [
                [
                    persist.tile(
                        [128, 512], f32r, tag=f"attr{c}_{qc}", name=f"attr{c}_{qc}"
                    )
                    for qc in range(2)
                ]
                for c in range(4)
            ]  # att^T normalized, per (2-head chunk, query half)
            bqk_t = persist.tile([128, 8], f32, tag="bqk")
            bv_t = persist.tile([128, 512], f32, tag="bv")



            with tc.tile_pool(name="ph1", bufs=1) as ph1:
                xTh = [
                    [
                        ph1.tile(
                            [128, 512], f32r, tag=f"xT{k}_{si}", name=f"xT{k}_{si}"
                        )
                        for si in range(2)
                    ]
                    for k in range(8)
                ]
                # wqk column strips [partition, ko, col]; host pre-tiled
                wqs = [
                    ph1.tile([128, 8, 128], f32r, tag=f"wq{cc}", name=f"wq{cc}")
                    for cc in range(8)
                ]
                wv = [
                    ph1.tile([128, 512], f32r, tag=f"wv{k}", name=f"wv{k}")
                    for k in range(8)
                ]
                # ones column of V-hat
                nc.vector.memset(vst[:, :, :, 64], 1.0)
                # PE warmup: throwaway matmuls fill the DMA-paced startup ramp
                # and keep the PE p-state/HAM warm before real work arrives
                wu = ph1.tile([128, 512], dt.bfloat16, tag="wu")
                nc.vector.memset(wu[:], 0.0)
                pwu = psav.tile([65, 512], f32, tag="av", name="pwu")

                def emit_warmup(n):
                    for _ in range(n):
                        nc.tensor.matmul(
                            pwu[:], wu[:, 0:65], wu[:], start=True, stop=True
                        )
                # DMA priority (transfers are serialized): the bytes needed
                # for pair-0 qc=0 attention go first, then the rest in use order
                nc.sync.dma_start(bqk_t[:], bqk_d)
                nc.sync.dma_start(wqs[0][:], wqk_d[0])
                nc.sync.dma_start(wqs[4][:], wqk_d[4])
                for k in range(8):
                    nc.sync.dma_start(
                        xTh[k][0][:], xT_d[k * 128:(k + 1) * 128, 0:512]
                    )
                for k in range(8):
                    nc.sync.dma_start(
                        xTh[k][1][:], xT_d[k * 128:(k + 1) * 128, 512:1024]
                    )
                nc.sync.dma_start(wqs[1][:], wqk_d[1])
                nc.sync.dma_start(wqs[5][:], wqk_d[5])
                for k in range(8):
                    nc.sync.dma_start(wv[k][:], wv_d[k * 128:(k + 1) * 128, :])
                nc.sync.dma_start(bv_t[:], bv_d)
                for p_ in range(2, 4):
                    nc.sync.dma_start(wqs[p_][:], wqk_d[p_])
                    nc.sync.dma_start(wqs[4 + p_][:], wqk_d[4 + p_])

                # ---- per head-pair software pipeline ----
                # Interleave: S^T+exp(p) runs on ACT while PE fills the gaps
                # with AV(p-1) groups (or Y_v for p=0) and the next pair's Yqk.
                yqs, yks = {}, {}

                def emit_yqk(p, warm=False):
                    for cc, tagn in ((p, "yq"), (4 + p, "yk")):
                        yt = yqk_pool.tile(
                            [128, N], f32r, tag=tagn, name=f"{tagn}{p}"
                        )
                        for s in range(2):
                            pq = ps_sm.tile(
                                [128, 512], f32, tag="sm", name=f"pq{cc}_{s}"
                            )
                            for k in range(8):
                                nc.tensor.matmul(
                                    pq[:],
                                    wqs[cc][:, k, :],
                                    xTh[k][s][:],
                                    start=(k == 0),
                                    stop=(k == 7),
                                )
                                if warm and k < 4:
                                    emit_warmup(1)
                            nc.vector.tensor_scalar_add(
                                yt[:, s * 512:(s + 1) * 512],
                                pq[:],
                                bqk_t[:, cc:cc + 1],
                            )
                        (yqs if tagn == "yq" else yks)[p] = yt

                def emit_yv_group(s):
                    pv = ps_sm.tile([128, 512], f32, tag="sm", name=f"pv{s}")
                    for k in range(8):
                        nc.tensor.matmul(
                            pv[:],
                            xTh[k][s // 4][:, (s % 4) * 128:(s % 4 + 1) * 128],
                            wv[k][:],
                            start=(k == 0),
                            stop=(k == 7),
                        )
                    nc.vector.tensor_add(
                        out=vst[:, s, :, 0:64],
                        in0=pv[:].rearrange("p (h d) -> p h d", h=8),
                        in1=bv_t[:].rearrange("p (h d) -> p h d", h=8),
                    )

                def emit_av_group(p, es, j, qc):
                    p0 = j * 64
                    h = 2 * p + j
                    pav = psav.tile([65, 512], f32, tag="av", name=f"pav{h}_{qc}")
                    for kc in range(8):
                        nc.tensor.matmul(
                            pav[:],
                            vst[:, kc, h, :],
                            es[(j, kc)][:, qc * 512:(qc + 1) * 512],
                            start=(kc == 0),
                            stop=(kc == 7),
                        )
                    rc = norm.tile([1, 512], f32, tag="rc", name=f"rc{h}{qc}")
                    nc.vector.reciprocal(rc[:], pav[64:65, :])
                    bc = norm.tile([64, 512], f32, tag="bc", name=f"bc{h}{qc}")
                    nc.gpsimd.partition_broadcast(bc[:], rc[0:1, :])
                    nc.vector.tensor_mul(
                        out=attr[p][qc][p0:p0 + 64, :],
                        in0=pav[0:64, :],
                        in1=bc[:],
                    )

                def emit_st_exp(p, filler):
                    """S^T+exp for pair p; `filler(kc)` emits PE work to
                    overlap the ACT-paced exp stream."""
                    yq, yk = yqs[p], yks[p]
                    es = {}
                    for kc in range(8):
                        psj = [
                            ps.tile([128, N], f32, tag="s", name=f"ps{p}_{j}_{kc}")
                            for j in range(2)
                        ]
                        for qc in range(2):
                            for j, p0 in ((0, 0), (1, 64)):
                                nc.tensor.matmul(
                                    psj[j][:, qc * 512:(qc + 1) * 512],
                                    yk[p0:p0 + 64, kc * 128:(kc + 1) * 128],
                                    yq[p0:p0 + 64, qc * 512:(qc + 1) * 512],
                                    start=True,
                                    stop=True,
                                )
                        for j in range(2):
                            e = es_pool.tile(
                                [128, N], dt.bfloat16, tag="es",
                                name=f"es{p}_{j}_{kc}",
                            )
                            nc.scalar.activation(
                                e[:], psj[j][:], Act.Exp, scale=SCALE
                            )
                            es[(j, kc)] = e
                        filler(kc)
                    return es

                emit_warmup(10)
                emit_yqk(0, warm=True)

                def filler0(kc):
                    if kc == 0:
                        emit_yqk(1)
                    if kc >= 2:
                        emit_yv_group(kc - 2)

                es_prev = emit_st_exp(0, filler0)
                emit_yv_group(6)
                emit_yv_group(7)
                prev_p = 0
                for p in range(1, 4):
                    avq = [(j, qc) for j in range(2) for qc in range(2)]

                    def filler(kc, _avq=avq, _pp=prev_p, _es=es_prev, _p=p):
                        if kc % 2 == 1 and _avq:
                            j, qc = _avq.pop(0)
                            emit_av_group(_pp, _es, j, qc)
                        if kc == 4 and _p + 1 < 4:
                            emit_yqk(_p + 1)

                    es_now = emit_st_exp(p, filler)
                    for j, qc in avq:
                        emit_av_group(prev_p, es_prev, j, qc)
                    es_prev, prev_p = es_now, p
            # ---- output projection ----
            with tc.tile_pool(name="proj", bufs=1) as proj:
                wp = [
                    proj.tile([128, 1024], f32r, tag=f"wp{c}", name=f"wp{c}")
                    for c in range(4)
                ]
                for c in range(4):
                    nc.sync.dma_start(wp[c][:], wp_d[c * 128:(c + 1) * 128, :])
                with tc.tile_pool(name="yo_pool", bufs=3) as yo_pool:

                    def emit_proj(st):
                        yo = yo_pool.tile([128, N], f32, tag="yo", name=f"yo{st}")
                        for oc in range(2):
                            pool_sel = ps_sm if (st + oc) % 2 == 0 else ps
                            po = pool_sel.tile(
                                [128, 512], f32,
                                tag="sm" if (st + oc) % 2 == 0 else "s",
                                name=f"po{st}_{oc}",
                            )
                            for c in range(4):
                                nc.tensor.matmul(
                                    po[:],
                                    attr[c][st // 4][
                                        :, (st % 4) * 128:(st % 4 + 1) * 128
                                    ],
                                    wp[c][:, oc * 512:(oc + 1) * 512],
                                    start=(c == 0),
                                    stop=(c == 3),
                                )
                            # alternate evict engine so the two halves overlap
                            if oc == 0:
                                nc.scalar.copy(
                                    yo[:, oc * 512:(oc + 1) * 512], po[:]
                                )
                            else:
                                nc.vector.tensor_copy(
                                    yo[:, oc * 512:(oc + 1) * 512], po[:]
                                )
                            nc.sync.dma_start(
                                y_d[st * 128:(st + 1) * 128,
                                    oc * 512:(oc + 1) * 512],
                                yo[:, oc * 512:(oc + 1) * 512],
                            )

                    for j in range(2):
                        emit_av_group(3, es_prev, j, 0)
                    for st in range(4):
                        emit_proj(st)
                    for j in range(2):
                        emit_av_group(3, es_prev, j, 1)
                    for st in range(4, 8):
                        emit_proj(st)

    nc.compile()
    return nc


def _get_nc():
    if "nc" not in _NC_CACHE:
        _NC_CACHE["nc"] = _build_bass()
    return _NC_CACHE["nc"]


def _shard_inputs(x, w_qkv, b_qkv, w_proj):
    """Build per-core input maps. core = 2*b + hh."""
    import ml_dtypes

    ones64 = np.ones((128, 64), dtype=ml_dtypes.bfloat16)

    in_maps = []
    for core in range(NCORES):
        b = core // 2
        hh = core % 2
        q_sl = slice(hh * 512, (hh + 1) * 512)
        k_sl = slice(1024 + hh * 512, 1024 + (hh + 1) * 512)
        v_sl = slice(2048 + hh * 512, 2048 + (hh + 1) * 512)

        xT = np.ascontiguousarray(x[b].T)
        wqk = np.concatenate([w_qkv[:, q_sl], w_qkv[:, k_sl]], axis=1)
        # [cc, p, ko, c]: tile (cc) is [partition, ko, col], contiguous per DMA
        wqk = np.ascontiguousarray(
            wqk.reshape(8, 128, 8, 128).transpose(2, 1, 0, 3)
        )
        wv = np.ascontiguousarray(w_qkv[:, v_sl])
        wp = np.ascontiguousarray(w_proj[hh * 512:(hh + 1) * 512, :])
        bqk = np.ascontiguousarray(
            np.concatenate([b_qkv[q_sl], b_qkv[k_sl]]).reshape(8, 128).T
        )
        bv = np.ascontiguousarray(np.broadcast_to(b_qkv[v_sl], (128, 512)))
        in_maps.append(
            {
                "xT": xT,
                "wqk": wqk,
                "wv": wv,
                "wp": wp,
                "bqk": bqk,
                "bv": bv,
                "ones64": ones64,
            }
        )
    return in_maps


def kernel(x, w_qkv, b_qkv, w_proj, b_proj):
    from concourse.bass_utils import run_bass_kernel_spmd

    x = np.asarray(x, dtype=np.float32)
    w_qkv = np.asarray(w_qkv, dtype=np.float32)
    b_qkv = np.asarray(b_qkv, dtype=np.float32)
    w_proj = np.asarray(w_proj, dtype=np.float32)
    b_proj = np.asarray(b_proj, dtype=np.float32)

    nc = _get_nc()
    in_maps = _shard_inputs(x, w_qkv, b_qkv, w_proj)
    res = run_bass_kernel_spmd(nc, in_maps, core_ids=list(range(NCORES)))

    out = np.empty((B, N, C), dtype=np.float32)
    for b in range(B):
        out[b] = res.results[2 * b]["y"] + res.results[2 * b + 1]["y"]
    out += b_proj
    return out


# revision 63
# speedup vs baseline: 1.0040x; 1.0040x over previous
"""Multi-head attention block (B=4, N=1024, C=1024, H=16, d=64) on 8 TRN2 cores.

Sharding: core = 2*b + hh  (batch b in 0..3, head-half hh in 0..1 -> 8 heads/core).
Each core computes the qkv projection for its 8 heads, attention, and a partial
output projection (its 512 rows of w_proj). Host sums the two partials per
batch and adds b_proj.

Per-core pipeline (dense matmuls in float32r -> 1 PE cycle/row; softmax
probabilities and V in bf16, accumulation always f32 in PSUM):
  - x^T fed from host, so Y_qk^T[cols,seq] = (W_qk strips).T @ x^T chunks gives
    q^T,k^T directly; Y_v[seq,vcols] = (x^T chunks).T @ W_v gives V naturally.
    Zero on-chip transposes.
  - per head pair (heads 2p, 2p+1 at partition bases 0/64): S^T[keys,q] =
    k^T.T @ q^T with K=64; the two heads' matmuls target disjoint PE row
    groups. exp on ACT (softmax max-subtraction safely skipped; the 1/sqrt(d)
    scale is folded into the exp), output bf16.
  - AV with V augmented by a ones column: one PSUM accumulation yields both
    att^T[64,q] and the softmax denominators (row 64). Normalize: DVE
    reciprocal (PSUM row), gpsimd partition_broadcast, DVE multiply -> att^T.
  - proj: out[seq,outfeat] = (att^T chunks).T @ w_proj chunks, DVE evict,
    DMA out.
The emission order software-pipelines the stages: while ACT runs pair p's
exp stream, PE retires AV groups of pair p-1 (Y_v for p=0) and the next
pair's QKV matmuls; the final AV groups interleave with the projection.
"""

import numpy as np

B = 4
N = 1024
C = 1024
H = 16
D = 64
NCORES = 8
SCALE = D ** -0.5


_NC_CACHE = {}


def _build_bass():
    import concourse.mybir as mybir
    from concourse import bacc
    from concourse.tile import TileContext

    dt = mybir.dt
    f32 = dt.float32
    f32r = dt.float32r
    Act = mybir.ActivationFunctionType

    nc = bacc.Bacc(
        "TRN2",
        target_bir_lowering=False,
        debug=False,
        num_devices=NCORES,
        num_swdge_queues=4,
    )

    # ---- DRAM I/O (per-core shards; host prepares layouts) ----
    xT_d = nc.dram_tensor("xT", [C, N], f32r, kind="ExternalInput").ap()
    wqk_d = nc.dram_tensor("wqk", [8, 128, 8, 128], f32r, kind="ExternalInput").ap()
    wv_d = nc.dram_tensor("wv", [C, 512], f32r, kind="ExternalInput").ap()
    wp_d = nc.dram_tensor("wp", [512, C], f32r, kind="ExternalInput").ap()
    bqk_d = nc.dram_tensor("bqk", [128, 8], f32, kind="ExternalInput").ap()
    bv_d = nc.dram_tensor("bv", [128, 512], f32, kind="ExternalInput").ap()
    ones_d = nc.dram_tensor("ones64", [128, 64], dt.bfloat16, kind="ExternalInput").ap()
    y_d = nc.dram_tensor("y", [N, C], f32, kind="ExternalOutput").ap()

    with TileContext(nc) as tc:
        with (
            tc.tile_pool(name="persist", bufs=1) as persist,
            tc.tile_pool(name="yqk_pool", bufs=3) as yqk_pool,
            tc.tile_pool(name="es_pool", bufs=22) as es_pool,
            tc.tile_pool(name="norm", bufs=3) as norm,
            tc.tile_pool(name="psum", bufs=2, space="PSUM") as ps,
            tc.tile_pool(name="psum_sm", bufs=2, space="PSUM") as ps_sm,
            tc.tile_pool(name="psav", bufs=2, space="PSUM") as psav,
        ):
            # persistent SBUF tensors
            vst = persist.tile([128, 8, 8, 65], dt.bfloat16, tag="vst")  # [keys128, s, h, d+1]
            attr = [
                [
                    persist.tile(
                        [128, 512], f32r, tag=f"attr{c}_{qc}", name=f"attr{c}_{qc}"
                    )
                    for qc in range(2)
                ]
                for c in range(4)
            ]  # att^T normalized, per (2-head chunk, query half)
            bqk_t = persist.tile([128, 8], f32, tag="bqk")
            bv_t = persist.tile([128, 512], f32, tag="bv")



            with tc.tile_pool(name="ph1", bufs=1) as ph1:
                xTh = [
                    [
                        ph1.tile(
                            [128, 512], f32r, tag=f"xT{k}_{si}", name=f"xT{k}_{si}"
                        )
                        for si in range(2)
                    ]
                    for k in range(8)
                ]
                # wqk column strips [partition, ko, col]; host pre-tiled
                wqs = [
                    ph1.tile([128, 8, 128], f32r, tag=f"wq{cc}", name=f"wq{cc}")
                    for cc in range(8)
                ]
                wv = [
                    ph1.tile([128, 512], f32r, tag=f"wv{k}", name=f"wv{k}")
                    for k in range(8)
                ]
                # ones column of V-hat
                nc.vector.memset(vst[:, :, :, 64], 1.0)
                # PE warmup: throwaway matmuls fill the DMA-paced startup ramp
                # and keep the PE p-state/HAM warm before real work arrives
                wu = ph1.tile([128, 512], dt.bfloat16, tag="wu")
                nc.vector.memset(wu[:], 0.0)
                pwu = psav.tile([65, 512], f32, tag="av", name="pwu")

                def emit_warmup(n):
                    for _ in range(n):
                        nc.tensor.matmul(
                            pwu[:], wu[:, 0:65], wu[:], start=True, stop=True
                        )
                # DMA priority (transfers are serialized): the bytes needed
                # for pair-0 qc=0 attention go first, then the rest in use order
                nc.sync.dma_start(bqk_t[:], bqk_d)
                nc.sync.dma_start(wqs[0][:], wqk_d[0])
                nc.sync.dma_start(wqs[4][:], wqk_d[4])
                for k in range(8):
                    nc.sync.dma_start(
                        xTh[k][0][:], xT_d[k * 128:(k + 1) * 128, 0:512]
                    )
                for k in range(8):
                    nc.sync.dma_start(
                        xTh[k][1][:], xT_d[k * 128:(k + 1) * 128, 512:1024]
                    )
                nc.sync.dma_start(wqs[1][:], wqk_d[1])
                nc.sync.dma_start(wqs[5][:], wqk_d[5])
                for k in range(8):
                    nc.sync.dma_start(wv[k][:], wv_d[k * 128:(k + 1) * 128, :])
                nc.sync.dma_start(bv_t[:], bv_d)
                for p_ in range(2, 4):
                    nc.sync.dma_start(wqs[p_][:], wqk_d[p_])
                    nc.sync.dma_start(wqs[4 + p_][:], wqk_d[4 + p_])

                # ---- per head-pair software pipeline ----
                # Interleave: S^T+exp(p) runs on ACT while PE fills the gaps
                # with AV(p-1) groups (or Y_v for p=0) and the next pair's Yqk.
                yqs, yks = {}, {}

                def emit_yqk(p, warm=False):
                    for cc, tagn in ((p, "yq"), (4 + p, "yk")):
                        yt = yqk_pool.tile(
                            [128, N], f32r, tag=tagn, name=f"{tagn}{p}"
                        )
                        for s in range(2):
                            pq = ps_sm.tile(
                                [128, 512], f32, tag="sm", name=f"pq{cc}_{s}"
                            )
                            for k in range(8):
                                nc.tensor.matmul(
                                    pq[:],
                                    wqs[cc][:, k, :],
                                    xTh[k][s][:],
                                    start=(k == 0),
                                    stop=(k == 7),
                                )
                                if warm and k < 4:
                                    emit_warmup(1)
                            nc.vector.tensor_scalar_add(
                                yt[:, s * 512:(s + 1) * 512],
                                pq[:],
                                bqk_t[:, cc:cc + 1],
                            )
                        (yqs if tagn == "yq" else yks)[p] = yt

                def emit_yv_group(s):
                    pv = ps_sm.tile([128, 512], f32, tag="sm", name=f"pv{s}")
                    for k in range(8):
                        nc.tensor.matmul(
                            pv[:],
                            xTh[k][s // 4][:, (s % 4) * 128:(s % 4 + 1) * 128],
                            wv[k][:],
                            start=(k == 0),
                            stop=(k == 7),
                        )
                    nc.vector.tensor_add(
                        out=vst[:, s, :, 0:64],
                        in0=pv[:].rearrange("p (h d) -> p h d", h=8),
                        in1=bv_t[:].rearrange("p (h d) -> p h d", h=8),
                    )

                def emit_av_group(p, es, j, qc):
                    p0 = j * 64
                    h = 2 * p + j
                    pav = psav.tile([65, 512], f32, tag="av", name=f"pav{h}_{qc}")
                    for kc in range(8):
                        nc.tensor.matmul(
                            pav[:],
                            vst[:, kc, h, :],
                            es[(j, kc)][:, qc * 512:(qc + 1) * 512],
                            start=(kc == 0),
                            stop=(kc == 7),
                        )
                    rc = norm.tile([1, 512], f32, tag="rc", name=f"rc{h}{qc}")
                    nc.vector.reciprocal(rc[:], pav[64:65, :])
                    bc = norm.tile([64, 512], f32, tag="bc", name=f"bc{h}{qc}")
                    nc.gpsimd.partition_broadcast(bc[:], rc[0:1, :])
                    nc.vector.tensor_mul(
                        out=attr[p][qc][p0:p0 + 64, :],
                        in0=pav[0:64, :],
                        in1=bc[:],
                    )

                def emit_st_exp(p, filler):
                    """S^T+exp for pair p; `filler(kc)` emits PE work to
                    overlap the ACT-paced exp stream."""
                    yq, yk = yqs[p], yks[p]
                    es = {}
                    for kc in range(8):
                        psj = [
                            ps.tile([128, N], f32, tag="s", name=f"ps{p}_{j}_{kc}")
                            for j in range(2)
                        ]
                        for qc in range(2):
                            for j, p0 in ((0, 0), (1, 64)):
                                nc.tensor.matmul(
                                    psj[j][:, qc * 512:(qc + 1) * 512],
                                    yk[p0:p0 + 64, kc * 128:(kc + 1) * 128],
                                    yq[p0:p0 + 64, qc * 512:(qc + 1) * 512],
                                    start=True,
                                    stop=True,
                                )
                        for j in range(2):
                            e = es_pool.tile(
                                [128, N], dt.bfloat16, tag="es",
                                name=f"es{p}_{j}_{kc}",
                            )
                            nc.scalar.activation(
                                e[:], psj[j][:], Act.Exp, scale=SCALE
                            )
                            es[(j, kc)] = e
                        filler(kc)
                    return es

                emit_warmup(10)
                emit_yqk(0, warm=True)

                def filler0(kc):
                    if kc == 0:
                        emit_yqk(1)
                    if kc >= 2:
                        emit_yv_group(kc - 2)

                es_prev = emit_st_exp(0, filler0)
                emit_yv_group(6)
                emit_yv_group(7)
                prev_p = 0
                for p in range(1, 4):
                    avq = [(j, qc) for j in range(2) for qc in range(2)]

                    def filler(kc, _avq=avq, _pp=prev_p, _es=es_prev, _p=p):
                        if kc % 2 == 1 and _avq:
                            j, qc = _avq.pop(0)
                            emit_av_group(_pp, _es, j, qc)
                        if kc == 4 and _p + 1 < 4:
                            emit_yqk(_p + 1)

                    es_now = emit_st_exp(p, filler)
                    for j, qc in avq:
                        emit_av_group(prev_p, es_prev, j, qc)
                    es_prev, prev_p = es_now, p
            # ---- output projection ----
            with tc.tile_pool(name="proj", bufs=1) as proj:
                wp = [
                    proj.tile([128, 1024], f32r, tag=f"wp{c}", name=f"wp{c}")
                    for c in range(4)
                ]
                for c in range(4):
                    nc.sync.dma_start(wp[c][:], wp_d[c * 128:(c + 1) * 128, :])
                with tc.tile_pool(name="yo_pool", bufs=5) as yo_pool:

                    def emit_proj(st):
                        yo = yo_pool.tile([128, N], f32, tag="yo", name=f"yo{st}")
                        for oc in range(2):
                            pool_sel = ps_sm if (st + oc) % 2 == 0 else ps
                            po = pool_sel.tile(
                                [128, 512], f32,
                                tag="sm" if (st + oc) % 2 == 0 else "s",
                                name=f"po{st}_{oc}",
                            )
                            for c in range(4):
                                nc.tensor.matmul(
                                    po[:],
                                    attr[c][st // 4][
                                        :, (st % 4) * 128:(st % 4 + 1) * 128
                                    ],
                                    wp[c][:, oc * 512:(oc + 1) * 512],
                                    start=(c == 0),
                                    stop=(c == 3),
                                )
                            # alternate evict engine so the two halves overlap
                            if oc == 0:
                                nc.scalar.copy(
                                    yo[:, oc * 512:(oc + 1) * 512], po[:]
                                )
                            else:
                                nc.vector.tensor_copy(
                                    yo[:, oc * 512:(oc + 1) * 512], po[:]
                                )
                            nc.sync.dma_start(
                                y_d[st * 128:(st + 1) * 128,
                                    oc * 512:(oc + 1) * 512],
                                yo[:, oc * 512:(oc + 1) * 512],
                            )

                    for j in range(2):
                        emit_av_group(3, es_prev, j, 0)
                    for st in range(4):
                        emit_proj(st)
                    for j in range(2):
                        emit_av_group(3, es_prev, j, 1)
                    for st in range(4, 8):
                        emit_proj(st)

    nc.compile()
    return nc


def _get_nc():
    if "nc" not in _NC_CACHE:
        _NC_CACHE["nc"] = _build_bass()
    return _NC_CACHE["nc"]


def _shard_inputs(x, w_qkv, b_qkv, w_proj):
    """Build per-core input maps. core = 2*b + hh."""
    import ml_dtypes

    ones64 = np.ones((128, 64), dtype=ml_dtypes.bfloat16)

    in_maps = []
    for core in range(NCORES):
        b = core // 2
        hh = core % 2
        q_sl = slice(hh * 512, (hh + 1) * 512)
        k_sl = slice(1024 + hh * 512, 1024 + (hh + 1) * 512)
        v_sl = slice(2048 + hh * 512, 2048 + (hh + 1) * 512)

        xT = np.ascontiguousarray(x[b].T)
        wqk = np.concatenate([w_qkv[:, q_sl], w_qkv[:, k_sl]], axis=1)
        # [cc, p, ko, c]: tile (cc) is [partition, ko, col], contiguous per DMA
        wqk = np.ascontiguousarray(
            wqk.reshape(8, 128, 8, 128).transpose(2, 1, 0, 3)
        )
        wv = np.ascontiguousarray(w_qkv[:, v_sl])
        wp = np.ascontiguousarray(w_proj[hh * 512:(hh + 1) * 512, :])
        bqk = np.ascontiguousarray(
            np.concatenate([b_qkv[q_sl], b_qkv[k_sl]]).reshape(8, 128).T
        )
        bv = np.ascontiguousarray(np.broadcast_to(b_qkv[v_sl], (128, 512)))
        in_maps.append(
            {
                "xT": xT,
                "wqk": wqk,
                "wv": wv,
                "wp": wp,
                "bqk": bqk,
                "bv": bv,
                "ones64": ones64,
            }
        )
    return in_maps


def kernel(x, w_qkv, b_qkv, w_proj, b_proj):
    from concourse.bass_utils import run_bass_kernel_spmd

    x = np.asarray(x, dtype=np.float32)
    w_qkv = np.asarray(w_qkv, dtype=np.float32)
    b_qkv = np.asarray(b_qkv, dtype=np.float32)
    w_proj = np.asarray(w_proj, dtype=np.float32)
    b_proj = np.asarray(b_proj, dtype=np.float32)

    nc = _get_nc()
    in_maps = _shard_inputs(x, w_qkv, b_qkv, w_proj)
    res = run_bass_kernel_spmd(nc, in_maps, core_ids=list(range(NCORES)))

    out = np.empty((B, N, C), dtype=np.float32)
    for b in range(B):
        out[b] = res.results[2 * b]["y"] + res.results[2 * b + 1]["y"]
    out += b_proj
    return out
